# revision 49
# baseline (speedup 1.0000x reference)
"""Trainium2 Bass kernel for nn_Attention_10582799417937 (v5).

Data-parallel over batch (32 -> 4 per core x 8 cores), weights replicated.
Per-core pipeline (per batch):
  depthwise 3x3 convs run entirely on the PE as fp8-e4m3 DoubleRow diag
  matmuls (two taps contracted per pass).  Stride-1 conv reads flat
  contiguous windows of the host-padded [34,34] image (junk columns
  dropped at eviction); stride-2 conv reads host-deinterleaved parity
  planes [4,17,17] so every tap window is contiguous.
  -> pointwise projections (PE matmuls, bf16)
  -> attention computed transposed: dotsT[j,i] = k_h^T . q_h^T; softmax
     denominators via ones-mask matmuls on PE, normalize on DVE.
  -> output projection (PE) -> bf16 DRAM (cast to f32 on host).
All BN affine and the V-path bias are folded into weights on the host.
"""
import sys
import numpy as np
import ml_dtypes

sys.path.insert(0, "/opt/trn_rl_repo")

import concourse.bass as bass
import concourse.mybir as mybir
import concourse.tile as tile
from concourse import bacc
from concourse.ap import AP
from concourse.bass_utils import run_bass_kernel_spmd

# ---- problem constants (hardcoded per spec) ----
B, C, H, W = 32, 384, 32, 32
HEADS, D = 6, 64
INNER = HEADS * D          # 384
SCALE = D ** -0.5
EPS = 1e-5
N_CORES = 8
B_LOC = B // N_CORES       # 4
HW = H * W                 # 1024
HK, WK = H // 2, W // 2
JK = HK * WK               # 256
KC = C // 128              # 3 channel chunks
MC = INNER // 128          # 3 inner chunks (also head pairs)
NPAIR = HEADS // 2         # 3

BF16 = mybir.dt.bfloat16
FP8 = mybir.dt.float8e4
F32 = mybir.dt.float32
AL = mybir.AluOpType
AF = mybir.ActivationFunctionType
DR = mybir.MatmulPerfMode.DoubleRow

TAPSCALE = 16.0            # keep fp8 tap values out of subnormal range

# s1 conv: padded image [34,34] stored TWICE per kc (second copy at
# +IMG2), so every DoubleRow tap-pair delta is a multiple of 16 (the HW
# requirement for the fast fp8 path; see checkMatmultPerfMode).
# Pair (t0 in copy1, t1 in copy2): delta = IMG2 + off(t1) - off(t0).
PAD = 34
PADN = PAD * PAD           # 1156
IMG2 = 0                   # single image copy (fast DR path never engages)
S1_PAIRS = [((0, 0), (0, 2)), ((1, 0), (1, 2)), ((2, 0), (2, 2)),
            ((0, 1), (2, 1))]
S1_SINGLE = (1, 1)
X1N = PADN                 # 1156
# s1 out-row chunks (row0, nrows); out free = nrows*34, junk at m%34>=32
# (balanced chunks avoid the overhead-dominated 2-row runt matmuls)
S1_CHUNKS = [(0, 11), (11, 11), (22, 10)]

# s2 conv: parity planes [rp, cp] each [17,17]
PLN = 17 * 17              # 289
X2N = 4 * PLN              # 1156
S2_PAIRS = [((0, 0), (0, 2)), ((1, 0), (1, 2)), ((2, 0), (2, 2)),
            ((0, 1), (2, 1))]
S2_SINGLE = (1, 1)
XBLK = X1N + X2N           # per-kc block in the xp tile


def _s1_off(dy, dx, row0):
    return (row0 + dy) * PAD + dx


def _s2_off(dy, dx):
    plane = (dy & 1) * 2 + (dx & 1)
    return plane * PLN + (1 if dy == 2 else 0) * 17 + (1 if dx == 2 else 0)


def _win(tile_ap, off, delta, n):
    """[128, 2, n] view at element offset `off`, k-tile stride `delta`."""
    a = tile_ap
    return AP(a.tensor, a.offset + off, [list(a.ap[0]), [delta, 2], [1, n]])


def build_nc():
    nc = bacc.Bacc(None, target_bir_lowering=False)
    x1_ext = nc.declare_dram_parameter("x1", [B_LOC, C, X1N], FP8, False)
    x2_ext = nc.declare_dram_parameter("x2", [B_LOC, C, X2N], FP8, False)
    aq_ext = nc.declare_dram_parameter("aq", [C, INNER], BF16, False)
    ak_ext = nc.declare_dram_parameter("ak", [C, INNER], BF16, False)
    av_ext = nc.declare_dram_parameter("av", [C, INNER], BF16, False)
    w2_ext = nc.declare_dram_parameter("w2", [INNER, C], BF16, False)
    qd_ext = nc.declare_dram_parameter("qdiag", [KC * 128, 4 * 256 + 128], FP8, False)
    kd_ext = nc.declare_dram_parameter("kvdiag", [KC * 128, 4 * 256 + 128], FP8, False)
    bq_ext = nc.declare_dram_parameter("bq", [INNER, 1], F32, False)
    bk_ext = nc.declare_dram_parameter("bk", [INNER, 1], F32, False)
    b2_ext = nc.declare_dram_parameter("b2", [C, 1], F32, False)
    out_ext = nc.declare_dram_parameter("out", [B_LOC, C, H, W], BF16, True)

    from contextlib import ExitStack
    with tile.TileContext(nc) as tc, ExitStack() as ctx:
        wpool = ctx.enter_context(tc.tile_pool(name="weights", bufs=1))
        xpool = ctx.enter_context(tc.tile_pool(name="xp", bufs=3))
        y1pool = ctx.enter_context(tc.tile_pool(name="y1", bufs=3))
        y2pool = ctx.enter_context(tc.tile_pool(name="y2", bufs=3))
        qpool = ctx.enter_context(tc.tile_pool(name="q", bufs=2))
        kpool = ctx.enter_context(tc.tile_pool(name="k", bufs=2))
        vpool = ctx.enter_context(tc.tile_pool(name="v", bufs=2))
        epool = ctx.enter_context(tc.tile_pool(name="et", bufs=16))
        spool = ctx.enter_context(tc.tile_pool(name="es", bufs=3))
        rpool = ctx.enter_context(tc.tile_pool(name="recip", bufs=3))
        opool = ctx.enter_context(tc.tile_pool(name="outT", bufs=2))
        fpool = ctx.enter_context(tc.tile_pool(name="fin", bufs=3))
        ps2 = ctx.enter_context(tc.tile_pool(name="ps2", bufs=3, space="PSUM"))
        ps1 = ctx.enter_context(tc.tile_pool(name="ps1", bufs=2, space="PSUM"))

        # ---- x for batch 0 first (critical path), then weights ----
        def dma_x(b):
            """Per-kc DMAs on the (otherwise idle) gpsimd queue so the first
            conv can start after 1/3 arrives and x never queues behind
            weight loads."""
            xp = xpool.tile([128, KC * XBLK], FP8, tag="xp", name="xp")
            for kc_ in range(KC):
                if b == 0 and kc_ == 0:
                    # split so conv chunk 0 starts after the first rows land
                    for lo, hi in ((0, 476), (476, 816), (816, X1N)):
                        nc.gpsimd.dma_start(
                            xp[:, lo:hi], x1_ext[b, 0:128, lo:hi])
                else:
                    nc.gpsimd.dma_start(
                        xp[:, kc_ * XBLK:kc_ * XBLK + X1N],
                        x1_ext[b, kc_ * 128:(kc_ + 1) * 128, :])
                nc.gpsimd.dma_start(
                    xp[:, kc_ * XBLK + X1N:(kc_ + 1) * XBLK],
                    x2_ext[b, kc_ * 128:(kc_ + 1) * 128, :])
            return xp

        xp0 = dma_x(0)

        def wload(ext, i, shape, dtype, tag):
            t = wpool.tile(shape, dtype, tag=f"{tag}{i}", name=f"{tag}{i}")
            nc.sync.dma_start(t[:], ext[i * 128:(i + 1) * 128, :])
            return t

        qdg = [wload(qd_ext, i, [128, 4 * 256 + 128], FP8, "qdg") for i in range(KC)]
        kdg = [wload(kd_ext, i, [128, 4 * 256 + 128], FP8, "kdg") for i in range(KC)]
        aq_sb = [wload(aq_ext, i, [128, INNER], BF16, "aq") for i in range(KC)]
        ak_sb = [wload(ak_ext, i, [128, INNER], BF16, "ak") for i in range(KC)]
        av_sb = [wload(av_ext, i, [128, INNER], BF16, "av") for i in range(KC)]
        w2_sb = [wload(w2_ext, i, [128, C], BF16, "w2") for i in range(MC)]
        bq_sb = [wload(bq_ext, i, [128, 1], F32, "bq") for i in range(MC)]
        bk_sb = [wload(bk_ext, i, [128, 1], F32, "bk") for i in range(MC)]
        b2_sb = [wload(b2_ext, i, [128, 1], F32, "b2") for i in range(MC)]

        # ones-mask for denominator matmuls (64 stationary columns; the two
        # head denominators land on different PE col groups and co-run)
        ones64 = wpool.tile([128, 64], BF16, tag="ones64", name="ones64")
        nc.gpsimd.memset(ones64[:], 1.0)

        def dr_w(dg, kc_, pi):
            return dg[kc_][:, pi * 256:(pi + 1) * 256].rearrange(
                "p (two m) -> p two m", two=2)

        def single_w(dg, kc_):
            return dg[kc_][:, 4 * 256:4 * 256 + 128]

        def conv_s1_chunk(xp, dg, kc_, ci, y1t, ev_act):
            """One row-chunk of the stride-1 conv for (batch, kc)."""
            base = kc_ * XBLK
            evrow = sum(S1_CHUNKS[i][1] for i in range(ci))
            for row0, nr in [S1_CHUNKS[ci]]:
                n = nr * PAD
                pst = ps1.tile([128, 512], F32, tag="ps1", name="c1")
                for pi, (t0, t1) in enumerate(S1_PAIRS):
                    o0 = base + _s1_off(*t0, row0)
                    delta = IMG2 + _s1_off(*t1, row0) - _s1_off(*t0, row0)
                    nc.tensor.matmul(pst[:, :n], dr_w(dg, kc_, pi),
                                     _win(xp[:], o0, delta, n),
                                     start=(pi == 0), stop=False,
                                     perf_mode=DR)
                o0 = base + _s1_off(*S1_SINGLE, row0)
                nc.tensor.matmul(pst[:, :n], single_w(dg, kc_),
                                 xp[:, o0:o0 + n], start=False, stop=True)
                src = pst[:, :n].rearrange("p (r c) -> p r c", c=PAD)[:, :, :32]
                dst = y1t[:, kc_ * HW + evrow * 32:kc_ * HW + (evrow + nr) * 32]
                dstv = dst.rearrange("p (r c) -> p r c", c=32)
                if ev_act:
                    nc.scalar.activation(dstv, src, AF.Copy)
                else:
                    nc.vector.tensor_copy(dstv, src)
                evrow += nr

        def conv_s1(xp, dg, kc_, y1t, ev_act):
            for ci in range(len(S1_CHUNKS)):
                conv_s1_chunk(xp, dg, kc_, ci, y1t, ev_act)

        def conv_s2(xp, dg, kc_, y2t, ev_act):
            """Stride-2 conv for one (batch, kc) on parity planes."""
            base = kc_ * XBLK + X1N
            n = 15 * 17 + 16   # 271
            pst = ps1.tile([128, 512], F32, tag="ps1", name="c2")
            for pi, (t0, t1) in enumerate(S2_PAIRS):
                o0 = base + _s2_off(*t0)
                delta = _s2_off(*t1) - _s2_off(*t0)
                nc.tensor.matmul(pst[:, :n], dr_w(dg, kc_, pi),
                                 _win(xp[:], o0, delta, n),
                                 start=(pi == 0), stop=False, perf_mode=DR)
            o0 = base + _s2_off(*S2_SINGLE)
            nc.tensor.matmul(pst[:, :n], single_w(dg, kc_),
                             xp[:, o0:o0 + n], start=False, stop=True)
            src = pst[:, :272].rearrange("p (r c) -> p r c", c=17)[:, :16, :16]
            dstv = y2t[:, kc_ * JK:(kc_ + 1) * JK].rearrange(
                "p (r c) -> p r c", c=16)
            if ev_act:
                nc.scalar.activation(dstv, src, AF.Copy)
            else:
                nc.vector.tensor_copy(dstv, src)

        def qpw(b, y1):
            qt = qpool.tile([128, MC * HW], BF16, tag="q", name="qsb")
            for mc_ in range(MC):
                ps = ps2.tile([128, 1024], F32, tag="ps2", name="psA")
                for n2 in range(2):
                    for kc_ in range(KC):
                        nc.tensor.matmul(
                            ps[:, n2 * 512:(n2 + 1) * 512],
                            aq_sb[kc_][:, mc_ * 128:(mc_ + 1) * 128],
                            y1[:, kc_ * HW + n2 * 512:kc_ * HW + n2 * 512 + 512],
                            start=(kc_ == 0), stop=(kc_ == KC - 1))
                if b == B_LOC - 1:
                    nc.scalar.activation(qt[:, mc_ * HW:(mc_ + 1) * HW], ps[:],
                                         AF.Identity, bias=bq_sb[mc_][:],
                                         scale=1.0)
                else:
                    nc.vector.tensor_scalar_add(
                        qt[:, mc_ * HW:(mc_ + 1) * HW], ps[:], bq_sb[mc_][:])
            return qt

        def kvpw(b, y2):
            kt = kpool.tile([128, MC * JK], BF16, tag="k", name="ksb")
            for mc_ in range(MC):
                ps = ps1.tile([128, JK], F32, tag="ps1", name="psBk")
                for kc_ in range(KC):
                    nc.tensor.matmul(
                        ps[:], ak_sb[kc_][:, mc_ * 128:(mc_ + 1) * 128],
                        y2[:, kc_ * JK:(kc_ + 1) * JK],
                        start=(kc_ == 0), stop=(kc_ == KC - 1))
                nc.scalar.activation(kt[:, mc_ * JK:(mc_ + 1) * JK], ps[:],
                                     AF.Identity, bias=bk_sb[mc_][:], scale=1.0)
            vt = vpool.tile([128, 2 * INNER], BF16, tag="v", name="vsb")
            for jc in range(2):
                ps = ps1.tile([128, INNER], F32, tag="ps1", name="psBv")
                for kc_ in range(KC):
                    nc.tensor.matmul(
                        ps[:], y2[:, kc_ * JK + jc * 128:kc_ * JK + jc * 128 + 128],
                        av_sb[kc_][:],
                        start=(kc_ == 0), stop=(kc_ == KC - 1))
                nc.scalar.activation(vt[:, jc * INNER:(jc + 1) * INNER], ps[:],
                                     AF.Copy)
            return kt, vt

        def dots_pair(b, qt, kt, p, midf=None):
            """Adjacent h0/h1 matmuls sit on different PE row groups and
            different PSUM banks.  midf emits PE filler between the two jc
            groups so exp(jc0) completes before its psum slots are reused."""
            et = [[None, None], [None, None]]
            for jc in range(2):
                if jc == 1 and midf is not None:
                    midf()
                psd = [ps2.tile([128, 1024], F32, tag="ps2", name="psd")
                       for _ in range(2)]
                for ic in range(2):
                    for h01 in range(2):
                        hs = h01 * 64
                        nc.tensor.matmul(
                            psd[h01][:, ic * 512:(ic + 1) * 512],
                            kt[hs:hs + 64, p * JK + jc * 128:p * JK + jc * 128 + 128],
                            qt[hs:hs + 64, p * HW + ic * 512:p * HW + ic * 512 + 512],
                            start=True, stop=True,
                            tile_position=(hs, 0))
                for h01 in range(2):
                    e = epool.tile([128, HW], BF16, tag="et", name="et")
                    nc.scalar.activation(e[:], psd[h01][:], AF.Exp, scale=SCALE)
                    et[h01][jc] = e
            return et

        def den_presum(b, et, p):
            """s_h = et[h][0] + et[h][1] on DVE (bf16, 2x mode)."""
            s = spool.tile([128, 2 * HW], BF16, tag="es", name="es")
            for h01 in range(2):
                nc.vector.tensor_tensor(
                    s[:, h01 * HW:(h01 + 1) * HW], et[h01][0][:], et[h01][1][:],
                    AL.add)
            return s

        def den_pair(b, es, p):
            rec = rpool.tile([128, HW], F32, tag="recip", name="recip")
            psn = ps2.tile([128, 1024], F32, tag="ps2", name="psn")
            for ic in range(2):
                for h01 in range(2):
                    hs = h01 * 64
                    nc.tensor.matmul(
                        psn[hs:hs + 64, ic * 512:(ic + 1) * 512], ones64[:],
                        es[:, h01 * HW + ic * 512:h01 * HW + ic * 512 + 512],
                        start=True, stop=True,
                        tile_position=(0, hs))
            nc.vector.reciprocal_approx_fast(out=rec[:], in_=psn[:])
            return rec

        def attnv_pair(b, et, vt, rec, p, ot):
            pso = ps2.tile([128, 1024], F32, tag="ps2", name="pso")
            for ic in range(2):
                for h01 in range(2):
                    hs = h01 * 64
                    for jc in range(2):
                        nc.tensor.matmul(
                            pso[hs:hs + 64, ic * 512:(ic + 1) * 512],
                            vt[:, jc * INNER + p * 128 + hs:
                               jc * INNER + p * 128 + hs + 64],
                            et[h01][jc][:][:, ic * 512:(ic + 1) * 512],
                            start=(jc == 0), stop=(jc == 1),
                            tile_position=(0, hs))
            nc.vector.tensor_tensor(ot[:, p * HW:(p + 1) * HW], pso[:], rec[:],
                                    AL.mult)

        def outproj(b, ot, mcs=None):
            for mc_ in (range(MC) if mcs is None else mcs):
                fin = fpool.tile([128, HW], BF16, tag="fin", name="fin")
                ps = ps2.tile([128, 1024], F32, tag="ps2", name="psE")
                for n2 in range(2):
                    for p in range(NPAIR):
                        nc.tensor.matmul(
                            ps[:, n2 * 512:(n2 + 1) * 512],
                            w2_sb[p][:, mc_ * 128:(mc_ + 1) * 128],
                            ot[:, p * HW + n2 * 512:p * HW + n2 * 512 + 512],
                            start=(p == 0), stop=(p == NPAIR - 1))
                if b % 2 == 0 or b == B_LOC - 1:
                    nc.scalar.activation(fin[:], ps[:], AF.Identity,
                                         bias=b2_sb[mc_][:], scale=1.0)
                else:
                    nc.vector.tensor_scalar_add(fin[:], ps[:], b2_sb[mc_][:])
                nc.sync.dma_start(
                    out_ext[b, mc_ * 128:(mc_ + 1) * 128, :, :],
                    fin[:].rearrange("p (h w) -> p h w", w=W))

        def new_y():
            y1 = y1pool.tile([128, KC * HW], BF16, tag="y1", name="y1")
            y2 = y2pool.tile([128, KC * JK], BF16, tag="y2", name="y2")
            return y1, y2

        # ---------------- schedule ----------------
        y1c, y2c = new_y()
        for kc_ in range(KC):
            conv_s1(xp0, qdg, kc_, y1c[:], ev_act=(kc_ == 1))
        for kc_ in range(KC):
            conv_s2(xp0, kdg, kc_, y2c[:], ev_act=(kc_ % 2 == 0))

        prev_ot = None
        for b in range(B_LOC):
            y1, y2 = y1c, y2c
            last = (b == B_LOC - 1)
            qt = qpw(b, y1[:])
            kt, vt = kvpw(b, y2[:])
            if prev_ot is not None and not last:
                outproj(b - 1, prev_ot[:])
            xpn = dma_x(b + 1) if b + 1 < B_LOC else None
            if xpn is not None:
                y1c, y2c = new_y()

            def midf(i):
                """PE filler inside dots pair i: first two conv chunks of
                the next batch's kc=i (or outproj chunks on the last batch)."""
                def run():
                    if not last:
                        for ci in (0, 1):
                            conv_s1_chunk(xpn, qdg, i, ci, y1c[:],
                                          ev_act=(i == 1))
                    elif prev_ot is not None:
                        outproj(b - 1, prev_ot[:], mcs=[i])
                return run

            def filler(i):
                if not last:
                    if b == 0:
                        # batch 1's x DMA is still in flight during batch 0's
                        # dots; emitting conv there would stall the PE
                        for ci in (0, 1):
                            conv_s1_chunk(xpn, qdg, i, ci, y1c[:],
                                          ev_act=(i == 1))
                    conv_s1_chunk(xpn, qdg, i, 2, y1c[:], ev_act=(i == 1))

            mf = (lambda i: None) if b == 0 else midf

            ot = opool.tile([128, NPAIR * HW], BF16, tag="outT", name="outT")
            e0 = dots_pair(b, qt[:], kt[:], 0, midf=mf(0))
            s0 = den_presum(b, e0, 0)
            filler(0)
            e1 = dots_pair(b, qt[:], kt[:], 1, midf=mf(1))
            s1_ = den_presum(b, e1, 1)
            filler(1)
            rec = den_pair(b, s0[:], 0)
            attnv_pair(b, e0, vt[:], rec, 0, ot[:])
            e2 = dots_pair(b, qt[:], kt[:], 2, midf=mf(2))
            s2_ = den_presum(b, e2, 2)
            filler(2)
            rec1 = den_pair(b, s1_[:], 1)
            attnv_pair(b, e1, vt[:], rec1, 1, ot[:])
            if not last:
                for kc_ in range(KC):
                    conv_s2(xpn, kdg, kc_, y2c[:], ev_act=(kc_ % 2 == 0))
            rec2 = den_pair(b, s2_[:], 2)
            attnv_pair(b, e2, vt[:], rec2, 2, ot[:])
            prev_ot = ot
        outproj(B_LOC - 1, prev_ot[:])

    nc.compile()
    return nc


_NC_CACHE = None


def _get_nc():
    global _NC_CACHE
    if _NC_CACHE is None:
        _NC_CACHE = build_nc()
    return _NC_CACHE


def _prep_host(inputs):
    """Fold BN into pointwise weights; fold v-bias into final bias.
    Conv taps scaled by TAPSCALE into fp8; descaled inside A matrices."""
    f32 = np.float32
    bf16 = ml_dtypes.bfloat16
    fp8 = ml_dtypes.float8_e4m3

    inv_q = (inputs['q_gamma'] / np.sqrt(inputs['q_var'] + EPS)).astype(f32)
    sh_q = (inputs['q_beta'] - inputs['q_mean'] * inv_q).astype(f32)
    A_q = (inputs['q_pw'] * inv_q[None, :]).astype(f32) / TAPSCALE
    b_q = (inputs['q_pw'].astype(f32) @ sh_q).astype(f32)

    inv_kv = (inputs['kv_gamma'] / np.sqrt(inputs['kv_var'] + EPS)).astype(f32)
    sh_kv = (inputs['kv_beta'] - inputs['kv_mean'] * inv_kv).astype(f32)
    A_kv = (inputs['kv_pw'] * inv_kv[None, :]).astype(f32) / TAPSCALE
    b_kv = (inputs['kv_pw'].astype(f32) @ sh_kv).astype(f32)
    A_k, A_v = A_kv[:INNER], A_kv[INNER:]
    b_k, b_v = b_kv[:INNER], b_kv[INNER:]

    W2 = inputs['out_w'].astype(f32)
    b2 = (inputs['out_b'].astype(f32) + W2 @ b_v).astype(f32)

    def diag_blocks(taps, pairs, single):
        t8 = (taps * TAPSCALE).astype(fp8).astype(f32)
        out = np.zeros((KC * 128, 4 * 256 + 128), f32)
        for kc_ in range(KC):
            blk = slice(kc_ * 128, (kc_ + 1) * 128)
            for pi, (t0, t1) in enumerate(pairs):
                i0 = t0[0] * 3 + t0[1]
                i1 = t1[0] * 3 + t1[1]
                out[blk, pi * 256:pi * 256 + 128] = np.diag(t8[blk, i0])
                out[blk, pi * 256 + 128:pi * 256 + 256] = np.diag(t8[blk, i1])
            i1 = single[0] * 3 + single[1]
            out[blk, 4 * 256:] = np.diag(t8[blk, i1])
        return out.astype(fp8)

    qdiag = diag_blocks(inputs['q_dw'].reshape(C, 9).astype(f32),
                        S1_PAIRS, S1_SINGLE)
    kvdiag = diag_blocks(inputs['kv_dw'].reshape(C, 9).astype(f32),
                         S2_PAIRS, S2_SINGLE)

    x8 = inputs['x'].astype(fp8)
    xpad = np.zeros((B, C, PAD, PAD), fp8)
    xpad[:, :, 1:33, 1:33] = x8
    xflat = xpad.reshape(B, C, PADN)
    x1 = np.zeros((B, C, X1N), fp8)
    x1[:, :, :PADN] = xflat
    if IMG2:
        x1[:, :, IMG2:] = xflat
    planes = np.zeros((B, C, 2, 2, 17, 17), fp8)
    for rp in range(2):
        for cp in range(2):
            sub = xpad[:, :, rp::2, cp::2]
            planes[:, :, rp, cp, :sub.shape[2], :sub.shape[3]] = sub
    x2 = planes.reshape(B, C, X2N)

    return {
        'qdiag': qdiag,
        'kvdiag': kvdiag,
        'aq': np.ascontiguousarray(A_q.T).astype(bf16),
        'ak': np.ascontiguousarray(A_k.T).astype(bf16),
        'av': np.ascontiguousarray(A_v.T).astype(bf16),
        'w2': np.ascontiguousarray(W2.T).astype(bf16),
        'bq': b_q.reshape(INNER, 1),
        'bk': b_k.reshape(INNER, 1),
        'b2': b2.reshape(C, 1),
    }, x1, x2


def kernel(**inputs):
    inputs = {k: np.asarray(v) for k, v in inputs.items()}
    nc = _get_nc()
    wmap, x1, x2 = _prep_host(inputs)
    in_maps = []
    for c in range(N_CORES):
        m = dict(wmap)
        m['x1'] = np.ascontiguousarray(x1[c * B_LOC:(c + 1) * B_LOC])
        m['x2'] = np.ascontiguousarray(x2[c * B_LOC:(c + 1) * B_LOC])
        in_maps.append(m)
    res = run_bass_kernel_spmd(nc, in_maps, core_ids=list(range(N_CORES)))
    shards = [res.results[i]['out'] for i in range(N_CORES)]
    return np.concatenate(shards, axis=0).astype(np.float32)


# revision 50
# speedup vs baseline: 1.1734x; 1.1734x over previous
"""Trainium2 Bass kernel for nn_Attention_10582799417937 (v5).

Data-parallel over batch (32 -> 4 per core x 8 cores), weights replicated.
Per-core pipeline (per batch):
  depthwise 3x3 convs run entirely on the PE as fp8-e4m3 DoubleRow diag
  matmuls (two taps contracted per pass).  Stride-1 conv reads flat
  contiguous windows of the host-padded [34,34] image (junk columns
  dropped at eviction); stride-2 conv reads host-deinterleaved parity
  planes [4,17,17] so every tap window is contiguous.
  -> pointwise projections (PE matmuls, bf16)
  -> attention computed transposed: dotsT[j,i] = k_h^T . q_h^T; softmax
     denominators via ones-mask matmuls on PE, normalize on DVE.
  -> output projection (PE) -> bf16 DRAM (cast to f32 on host).
All BN affine and the V-path bias are folded into weights on the host.
"""
import sys
import numpy as np
import ml_dtypes

sys.path.insert(0, "/opt/trn_rl_repo")

import concourse.bass as bass
import concourse.mybir as mybir
import concourse.tile as tile
from concourse import bacc
from concourse.ap import AP
from concourse.bass_utils import run_bass_kernel_spmd

# ---- problem constants (hardcoded per spec) ----
B, C, H, W = 32, 384, 32, 32
HEADS, D = 6, 64
INNER = HEADS * D          # 384
SCALE = D ** -0.5
EPS = 1e-5
N_CORES = 8
B_LOC = B // N_CORES       # 4
HW = H * W                 # 1024
HK, WK = H // 2, W // 2
JK = HK * WK               # 256
KC = C // 128              # 3 channel chunks
MC = INNER // 128          # 3 inner chunks (also head pairs)
NPAIR = HEADS // 2         # 3

BF16 = mybir.dt.bfloat16
FP8 = mybir.dt.float8e4
F32 = mybir.dt.float32
AL = mybir.AluOpType
AF = mybir.ActivationFunctionType
DR = mybir.MatmulPerfMode.DoubleRow

TAPSCALE = 16.0            # keep fp8 tap values out of subnormal range

# s1 conv: padded image [34,34] stored TWICE per kc (second copy at
# +IMG2), so every DoubleRow tap-pair delta is a multiple of 16 (the HW
# requirement for the fast fp8 path; see checkMatmultPerfMode).
# Pair (t0 in copy1, t1 in copy2): delta = IMG2 + off(t1) - off(t0).
PAD = 34
PADN = PAD * PAD           # 1156
IMG2 = 0                   # single image copy (fast DR path never engages)
S1_PAIRS = [((0, 0), (0, 2)), ((1, 0), (1, 2)), ((2, 0), (2, 2)),
            ((0, 1), (2, 1))]
S1_SINGLE = (1, 1)
X1N = PADN                 # 1156
# s1 out-row chunks (row0, nrows); out free = nrows*34, junk at m%34>=32
# (balanced chunks avoid the overhead-dominated 2-row runt matmuls)
S1_CHUNKS = [(0, 11), (11, 11), (22, 10)]

# s2 conv: parity planes [rp, cp] each [17,17]
PLN = 17 * 17              # 289
X2N = 4 * PLN              # 1156
S2_PAIRS = [((0, 0), (0, 2)), ((1, 0), (1, 2)), ((2, 0), (2, 2)),
            ((0, 1), (2, 1))]
S2_SINGLE = (1, 1)
XBLK = X1N + X2N           # per-kc block in the xp tile


def _s1_off(dy, dx, row0):
    return (row0 + dy) * PAD + dx


def _s2_off(dy, dx):
    plane = (dy & 1) * 2 + (dx & 1)
    return plane * PLN + (1 if dy == 2 else 0) * 17 + (1 if dx == 2 else 0)


def _win(tile_ap, off, delta, n):
    """[128, 2, n] view at element offset `off`, k-tile stride `delta`."""
    a = tile_ap
    return AP(a.tensor, a.offset + off, [list(a.ap[0]), [delta, 2], [1, n]])


def build_nc():
    nc = bacc.Bacc(None, target_bir_lowering=False)
    x1_ext = nc.declare_dram_parameter("x1", [B_LOC, C, X1N], FP8, False)
    x2_ext = nc.declare_dram_parameter("x2", [B_LOC, C, X2N], FP8, False)
    aq_ext = nc.declare_dram_parameter("aq", [C, INNER], BF16, False)
    ak_ext = nc.declare_dram_parameter("ak", [C, INNER], BF16, False)
    av_ext = nc.declare_dram_parameter("av", [C, INNER], BF16, False)
    w2_ext = nc.declare_dram_parameter("w2", [INNER, C], BF16, False)
    qd_ext = nc.declare_dram_parameter("qdiag", [KC * 128, 4 * 256 + 128], FP8, False)
    kd_ext = nc.declare_dram_parameter("kvdiag", [KC * 128, 4 * 256 + 128], FP8, False)
    bq_ext = nc.declare_dram_parameter("bq", [INNER, 1], F32, False)
    bk_ext = nc.declare_dram_parameter("bk", [INNER, 1], F32, False)
    b2_ext = nc.declare_dram_parameter("b2", [C, 1], F32, False)
    out_ext = nc.declare_dram_parameter("out", [B_LOC, C, H, W], BF16, True)

    from contextlib import ExitStack
    with tile.TileContext(nc) as tc, ExitStack() as ctx:
        wpool = ctx.enter_context(tc.tile_pool(name="weights", bufs=1))
        xpool = ctx.enter_context(tc.tile_pool(name="xp", bufs=3))
        y1pool = ctx.enter_context(tc.tile_pool(name="y1", bufs=3))
        y2pool = ctx.enter_context(tc.tile_pool(name="y2", bufs=3))
        qpool = ctx.enter_context(tc.tile_pool(name="q", bufs=2))
        kpool = ctx.enter_context(tc.tile_pool(name="k", bufs=2))
        vpool = ctx.enter_context(tc.tile_pool(name="v", bufs=2))
        epool = ctx.enter_context(tc.tile_pool(name="et", bufs=14))
        spool = ctx.enter_context(tc.tile_pool(name="es", bufs=3))
        rpool = ctx.enter_context(tc.tile_pool(name="recip", bufs=3))
        opool = ctx.enter_context(tc.tile_pool(name="outT", bufs=2))
        fpool = ctx.enter_context(tc.tile_pool(name="fin", bufs=3))
        ps2 = ctx.enter_context(tc.tile_pool(name="ps2", bufs=3, space="PSUM"))
        ps1 = ctx.enter_context(tc.tile_pool(name="ps1", bufs=2, space="PSUM"))

        # ---- x for batch 0 first (critical path), then weights ----
        def dma_x(b):
            """Per-kc DMAs on the (otherwise idle) gpsimd queue so the first
            conv can start after 1/3 arrives and x never queues behind
            weight loads."""
            xp = xpool.tile([128, KC * XBLK], FP8, tag="xp", name="xp")
            for kc_ in range(KC):
                if b == 0 and kc_ == 0:
                    # split so conv chunk 0 starts after the first rows land
                    for lo, hi in ((0, 476), (476, 816), (816, X1N)):
                        nc.gpsimd.dma_start(
                            xp[:, lo:hi], x1_ext[b, 0:128, lo:hi])
                else:
                    nc.gpsimd.dma_start(
                        xp[:, kc_ * XBLK:kc_ * XBLK + X1N],
                        x1_ext[b, kc_ * 128:(kc_ + 1) * 128, :])
                nc.gpsimd.dma_start(
                    xp[:, kc_ * XBLK + X1N:(kc_ + 1) * XBLK],
                    x2_ext[b, kc_ * 128:(kc_ + 1) * 128, :])
            return xp

        xp0 = dma_x(0)

        def wload(ext, i, shape, dtype, tag):
            t = wpool.tile(shape, dtype, tag=f"{tag}{i}", name=f"{tag}{i}")
            nc.sync.dma_start(t[:], ext[i * 128:(i + 1) * 128, :])
            return t

        qdg = [wload(qd_ext, i, [128, 4 * 256 + 128], FP8, "qdg") for i in range(KC)]
        kdg = [wload(kd_ext, i, [128, 4 * 256 + 128], FP8, "kdg") for i in range(KC)]
        aq_sb = [wload(aq_ext, i, [128, INNER], BF16, "aq") for i in range(KC)]
        ak_sb = [wload(ak_ext, i, [128, INNER], BF16, "ak") for i in range(KC)]
        av_sb = [wload(av_ext, i, [128, INNER], BF16, "av") for i in range(KC)]
        w2_sb = [wload(w2_ext, i, [128, C], BF16, "w2") for i in range(MC)]
        bq_sb = [wload(bq_ext, i, [128, 1], F32, "bq") for i in range(MC)]
        bk_sb = [wload(bk_ext, i, [128, 1], F32, "bk") for i in range(MC)]
        b2_sb = [wload(b2_ext, i, [128, 1], F32, "b2") for i in range(MC)]

        # ones-mask for denominator matmuls (64 stationary columns; the two
        # head denominators land on different PE col groups and co-run)
        ones64 = wpool.tile([128, 64], BF16, tag="ones64", name="ones64")
        nc.gpsimd.memset(ones64[:], 1.0)

        def dr_w(dg, kc_, pi):
            return dg[kc_][:, pi * 256:(pi + 1) * 256].rearrange(
                "p (two m) -> p two m", two=2)

        def single_w(dg, kc_):
            return dg[kc_][:, 4 * 256:4 * 256 + 128]

        def conv_s1_chunk(xp, dg, kc_, ci, y1t, ev_act):
            """One row-chunk of the stride-1 conv for (batch, kc)."""
            base = kc_ * XBLK
            evrow = sum(S1_CHUNKS[i][1] for i in range(ci))
            for row0, nr in [S1_CHUNKS[ci]]:
                n = nr * PAD
                pst = ps1.tile([128, 512], F32, tag="ps1", name="c1")
                for pi, (t0, t1) in enumerate(S1_PAIRS):
                    o0 = base + _s1_off(*t0, row0)
                    delta = IMG2 + _s1_off(*t1, row0) - _s1_off(*t0, row0)
                    nc.tensor.matmul(pst[:, :n], dr_w(dg, kc_, pi),
                                     _win(xp[:], o0, delta, n),
                                     start=(pi == 0), stop=False,
                                     perf_mode=DR)
                o0 = base + _s1_off(*S1_SINGLE, row0)
                nc.tensor.matmul(pst[:, :n], single_w(dg, kc_),
                                 xp[:, o0:o0 + n], start=False, stop=True)
                src = pst[:, :n].rearrange("p (r c) -> p r c", c=PAD)[:, :, :32]
                dst = y1t[:, kc_ * HW + evrow * 32:kc_ * HW + (evrow + nr) * 32]
                dstv = dst.rearrange("p (r c) -> p r c", c=32)
                if ev_act:
                    nc.scalar.activation(dstv, src, AF.Copy)
                else:
                    nc.vector.tensor_copy(dstv, src)
                evrow += nr

        def conv_s1(xp, dg, kc_, y1t, ev_act):
            for ci in range(len(S1_CHUNKS)):
                conv_s1_chunk(xp, dg, kc_, ci, y1t, ev_act)

        def conv_s2(xp, dg, kc_, y2t, ev_act):
            """Stride-2 conv for one (batch, kc) on parity planes."""
            base = kc_ * XBLK + X1N
            n = 15 * 17 + 16   # 271
            pst = ps1.tile([128, 512], F32, tag="ps1", name="c2")
            for pi, (t0, t1) in enumerate(S2_PAIRS):
                o0 = base + _s2_off(*t0)
                delta = _s2_off(*t1) - _s2_off(*t0)
                nc.tensor.matmul(pst[:, :n], dr_w(dg, kc_, pi),
                                 _win(xp[:], o0, delta, n),
                                 start=(pi == 0), stop=False, perf_mode=DR)
            o0 = base + _s2_off(*S2_SINGLE)
            nc.tensor.matmul(pst[:, :n], single_w(dg, kc_),
                             xp[:, o0:o0 + n], start=False, stop=True)
            src = pst[:, :272].rearrange("p (r c) -> p r c", c=17)[:, :16, :16]
            dstv = y2t[:, kc_ * JK:(kc_ + 1) * JK].rearrange(
                "p (r c) -> p r c", c=16)
            if ev_act:
                nc.scalar.activation(dstv, src, AF.Copy)
            else:
                nc.vector.tensor_copy(dstv, src)

        def qpw(b, y1):
            qt = qpool.tile([128, MC * HW], BF16, tag="q", name="qsb")
            for mc_ in range(MC):
                ps = ps2.tile([128, 1024], F32, tag="ps2", name="psA")
                for n2 in range(2):
                    for kc_ in range(KC):
                        nc.tensor.matmul(
                            ps[:, n2 * 512:(n2 + 1) * 512],
                            aq_sb[kc_][:, mc_ * 128:(mc_ + 1) * 128],
                            y1[:, kc_ * HW + n2 * 512:kc_ * HW + n2 * 512 + 512],
                            start=(kc_ == 0), stop=(kc_ == KC - 1))
                if b == B_LOC - 1:
                    nc.scalar.activation(qt[:, mc_ * HW:(mc_ + 1) * HW], ps[:],
                                         AF.Identity, bias=bq_sb[mc_][:],
                                         scale=1.0)
                else:
                    nc.vector.tensor_scalar_add(
                        qt[:, mc_ * HW:(mc_ + 1) * HW], ps[:], bq_sb[mc_][:])
            return qt

        def kvpw(b, y2):
            kt = kpool.tile([128, MC * JK], BF16, tag="k", name="ksb")
            for mc_ in range(MC):
                ps = ps1.tile([128, JK], F32, tag="ps1", name="psBk")
                for kc_ in range(KC):
                    nc.tensor.matmul(
                        ps[:], ak_sb[kc_][:, mc_ * 128:(mc_ + 1) * 128],
                        y2[:, kc_ * JK:(kc_ + 1) * JK],
                        start=(kc_ == 0), stop=(kc_ == KC - 1))
                nc.scalar.activation(kt[:, mc_ * JK:(mc_ + 1) * JK], ps[:],
                                     AF.Identity, bias=bk_sb[mc_][:], scale=1.0)
            vt = vpool.tile([128, 2 * INNER], BF16, tag="v", name="vsb")
            for jc in range(2):
                ps = ps1.tile([128, INNER], F32, tag="ps1", name="psBv")
                for kc_ in range(KC):
                    nc.tensor.matmul(
                        ps[:], y2[:, kc_ * JK + jc * 128:kc_ * JK + jc * 128 + 128],
                        av_sb[kc_][:],
                        start=(kc_ == 0), stop=(kc_ == KC - 1))
                nc.scalar.activation(vt[:, jc * INNER:(jc + 1) * INNER], ps[:],
                                     AF.Copy)
            return kt, vt

        def dots_pair(b, qt, kt, p, midf=None):
            """Adjacent h0/h1 matmuls sit on different PE row groups and
            different PSUM banks.  midf emits PE filler between the two jc
            groups so exp(jc0) completes before its psum slots are reused."""
            et = [[None, None], [None, None]]
            for jc in range(2):
                if jc == 1 and midf is not None:
                    midf()
                psd = [ps2.tile([128, 1024], F32, tag="ps2", name="psd")
                       for _ in range(2)]
                for ic in range(2):
                    for h01 in range(2):
                        hs = h01 * 64
                        nc.tensor.matmul(
                            psd[h01][:, ic * 512:(ic + 1) * 512],
                            kt[hs:hs + 64, p * JK + jc * 128:p * JK + jc * 128 + 128],
                            qt[hs:hs + 64, p * HW + ic * 512:p * HW + ic * 512 + 512],
                            start=True, stop=True,
                            tile_position=(hs, 0))
                for h01 in range(2):
                    e = epool.tile([128, HW], BF16, tag="et", name="et")
                    nc.scalar.activation(e[:], psd[h01][:], AF.Exp, scale=SCALE)
                    et[h01][jc] = e
            return et

        def den_presum(b, et, p):
            """s_h = et[h][0] + et[h][1] on DVE (bf16, 2x mode)."""
            s = spool.tile([128, 2 * HW], BF16, tag="es", name="es")
            for h01 in range(2):
                nc.vector.tensor_tensor(
                    s[:, h01 * HW:(h01 + 1) * HW], et[h01][0][:], et[h01][1][:],
                    AL.add)
            return s

        def den_pair(b, es, p):
            rec = rpool.tile([128, HW], F32, tag="recip", name="recip")
            psn = ps2.tile([128, 1024], F32, tag="ps2", name="psn")
            for ic in range(2):
                for h01 in range(2):
                    hs = h01 * 64
                    nc.tensor.matmul(
                        psn[hs:hs + 64, ic * 512:(ic + 1) * 512], ones64[:],
                        es[:, h01 * HW + ic * 512:h01 * HW + ic * 512 + 512],
                        start=True, stop=True,
                        tile_position=(0, hs))
            nc.vector.reciprocal_approx_fast(out=rec[:], in_=psn[:])
            return rec

        def attnv_pair(b, et, vt, rec, p, ot):
            pso = ps2.tile([128, 1024], F32, tag="ps2", name="pso")
            for ic in range(2):
                for h01 in range(2):
                    hs = h01 * 64
                    for jc in range(2):
                        nc.tensor.matmul(
                            pso[hs:hs + 64, ic * 512:(ic + 1) * 512],
                            vt[:, jc * INNER + p * 128 + hs:
                               jc * INNER + p * 128 + hs + 64],
                            et[h01][jc][:][:, ic * 512:(ic + 1) * 512],
                            start=(jc == 0), stop=(jc == 1),
                            tile_position=(0, hs))
            nc.vector.tensor_tensor(ot[:, p * HW:(p + 1) * HW], pso[:], rec[:],
                                    AL.mult)

        def outproj(b, ot, mcs=None):
            for mc_ in (range(MC) if mcs is None else mcs):
                fin = fpool.tile([128, HW], BF16, tag="fin", name="fin")
                ps = ps2.tile([128, 1024], F32, tag="ps2", name="psE")
                for n2 in range(2):
                    for p in range(NPAIR):
                        nc.tensor.matmul(
                            ps[:, n2 * 512:(n2 + 1) * 512],
                            w2_sb[p][:, mc_ * 128:(mc_ + 1) * 128],
                            ot[:, p * HW + n2 * 512:p * HW + n2 * 512 + 512],
                            start=(p == 0), stop=(p == NPAIR - 1))
                if b % 2 == 0 or b == B_LOC - 1:
                    nc.scalar.activation(fin[:], ps[:], AF.Identity,
                                         bias=b2_sb[mc_][:], scale=1.0)
                else:
                    nc.vector.tensor_scalar_add(fin[:], ps[:], b2_sb[mc_][:])
                nc.sync.dma_start(
                    out_ext[b, mc_ * 128:(mc_ + 1) * 128, :, :],
                    fin[:].rearrange("p (h w) -> p h w", w=W))

        def new_y():
            y1 = y1pool.tile([128, KC * HW], BF16, tag="y1", name="y1")
            y2 = y2pool.tile([128, KC * JK], BF16, tag="y2", name="y2")
            return y1, y2

        # ---------------- schedule ----------------
        y1c, y2c = new_y()
        for kc_ in range(KC):
            conv_s1(xp0, qdg, kc_, y1c[:], ev_act=(kc_ == 1))
        for kc_ in range(KC):
            conv_s2(xp0, kdg, kc_, y2c[:], ev_act=(kc_ % 2 == 0))

        prev_ot = None
        for b in range(B_LOC):
            y1, y2 = y1c, y2c
            last = (b == B_LOC - 1)
            qt = qpw(b, y1[:])
            kt, vt = kvpw(b, y2[:])
            if prev_ot is not None and not last:
                outproj(b - 1, prev_ot[:])
            xpn = dma_x(b + 1) if b + 1 < B_LOC else None
            if xpn is not None:
                y1c, y2c = new_y()

            def midf(i):
                """PE filler inside dots pair i: first two conv chunks of
                the next batch's kc=i (or outproj chunks on the last batch)."""
                def run():
                    if not last:
                        for ci in (0, 1):
                            conv_s1_chunk(xpn, qdg, i, ci, y1c[:],
                                          ev_act=(i == 1))
                    elif prev_ot is not None:
                        outproj(b - 1, prev_ot[:], mcs=[i])
                return run

            def filler(i):
                if not last:
                    if b == 0:
                        # batch 1's x DMA is still in flight during batch 0's
                        # dots; emitting conv there would stall the PE
                        for ci in (0, 1):
                            conv_s1_chunk(xpn, qdg, i, ci, y1c[:],
                                          ev_act=(i == 1))
                    conv_s1_chunk(xpn, qdg, i, 2, y1c[:], ev_act=(i == 1))

            mf = (lambda i: None) if b == 0 else midf

            ot = opool.tile([128, NPAIR * HW], BF16, tag="outT", name="outT")
            e0 = dots_pair(b, qt[:], kt[:], 0, midf=mf(0))
            s0 = den_presum(b, e0, 0)
            filler(0)
            e1 = dots_pair(b, qt[:], kt[:], 1, midf=mf(1))
            s1_ = den_presum(b, e1, 1)
            filler(1)
            rec = den_pair(b, s0[:], 0)
            attnv_pair(b, e0, vt[:], rec, 0, ot[:])
            e2 = dots_pair(b, qt[:], kt[:], 2, midf=mf(2))
            s2_ = den_presum(b, e2, 2)
            filler(2)
            rec1 = den_pair(b, s1_[:], 1)
            attnv_pair(b, e1, vt[:], rec1, 1, ot[:])
            if not last:
                for kc_ in range(KC):
                    conv_s2(xpn, kdg, kc_, y2c[:], ev_act=(kc_ % 2 == 0))
            rec2 = den_pair(b, s2_[:], 2)
            attnv_pair(b, e2, vt[:], rec2, 2, ot[:])
            prev_ot = ot
        outproj(B_LOC - 1, prev_ot[:])

    nc.compile()
    return nc


_NC_CACHE = None


def _get_nc():
    global _NC_CACHE
    if _NC_CACHE is None:
        _NC_CACHE = build_nc()
    return _NC_CACHE


def _prep_host(inputs):
    """Fold BN into pointwise weights; fold v-bias into final bias.
    Conv taps scaled by TAPSCALE into fp8; descaled inside A matrices."""
    f32 = np.float32
    bf16 = ml_dtypes.bfloat16
    fp8 = ml_dtypes.float8_e4m3

    inv_q = (inputs['q_gamma'] / np.sqrt(inputs['q_var'] + EPS)).astype(f32)
    sh_q = (inputs['q_beta'] - inputs['q_mean'] * inv_q).astype(f32)
    A_q = (inputs['q_pw'] * inv_q[None, :]).astype(f32) / TAPSCALE
    b_q = (inputs['q_pw'].astype(f32) @ sh_q).astype(f32)

    inv_kv = (inputs['kv_gamma'] / np.sqrt(inputs['kv_var'] + EPS)).astype(f32)
    sh_kv = (inputs['kv_beta'] - inputs['kv_mean'] * inv_kv).astype(f32)
    A_kv = (inputs['kv_pw'] * inv_kv[None, :]).astype(f32) / TAPSCALE
    b_kv = (inputs['kv_pw'].astype(f32) @ sh_kv).astype(f32)
    A_k, A_v = A_kv[:INNER], A_kv[INNER:]
    b_k, b_v = b_kv[:INNER], b_kv[INNER:]

    W2 = inputs['out_w'].astype(f32)
    b2 = (inputs['out_b'].astype(f32) + W2 @ b_v).astype(f32)

    def diag_blocks(taps, pairs, single):
        t8 = (taps * TAPSCALE).astype(fp8).astype(f32)
        out = np.zeros((KC * 128, 4 * 256 + 128), f32)
        for kc_ in range(KC):
            blk = slice(kc_ * 128, (kc_ + 1) * 128)
            for pi, (t0, t1) in enumerate(pairs):
                i0 = t0[0] * 3 + t0[1]
                i1 = t1[0] * 3 + t1[1]
                out[blk, pi * 256:pi * 256 + 128] = np.diag(t8[blk, i0])
                out[blk, pi * 256 + 128:pi * 256 + 256] = np.diag(t8[blk, i1])
            i1 = single[0] * 3 + single[1]
            out[blk, 4 * 256:] = np.diag(t8[blk, i1])
        return out.astype(fp8)

    qdiag = diag_blocks(inputs['q_dw'].reshape(C, 9).astype(f32),
                        S1_PAIRS, S1_SINGLE)
    kvdiag = diag_blocks(inputs['kv_dw'].reshape(C, 9).astype(f32),
                         S2_PAIRS, S2_SINGLE)

    x8 = inputs['x'].astype(fp8)
    xpad = np.zeros((B, C, PAD, PAD), fp8)
    xpad[:, :, 1:33, 1:33] = x8
    xflat = xpad.reshape(B, C, PADN)
    x1 = np.zeros((B, C, X1N), fp8)
    x1[:, :, :PADN] = xflat
    if IMG2:
        x1[:, :, IMG2:] = xflat
    planes = np.zeros((B, C, 2, 2, 17, 17), fp8)
    for rp in range(2):
        for cp in range(2):
            sub = xpad[:, :, rp::2, cp::2]
            planes[:, :, rp, cp, :sub.shape[2], :sub.shape[3]] = sub
    x2 = planes.reshape(B, C, X2N)

    return {
        'qdiag': qdiag,
        'kvdiag': kvdiag,
        'aq': np.ascontiguousarray(A_q.T).astype(bf16),
        'ak': np.ascontiguousarray(A_k.T).astype(bf16),
        'av': np.ascontiguousarray(A_v.T).astype(bf16),
        'w2': np.ascontiguousarray(W2.T).astype(bf16),
        'bq': b_q.reshape(INNER, 1),
        'bk': b_k.reshape(INNER, 1),
        'b2': b2.reshape(C, 1),
    }, x1, x2


def kernel(**inputs):
    inputs = {k: np.asarray(v) for k, v in inputs.items()}
    nc = _get_nc()
    wmap, x1, x2 = _prep_host(inputs)
    in_maps = []
    for c in range(N_CORES):
        m = dict(wmap)
        m['x1'] = np.ascontiguousarray(x1[c * B_LOC:(c + 1) * B_LOC])
        m['x2'] = np.ascontiguousarray(x2[c * B_LOC:(c + 1) * B_LOC])
        in_maps.append(m)
    res = run_bass_kernel_spmd(nc, in_maps, core_ids=list(range(N_CORES)))
    shards = [res.results[i]['out'] for i in range(N_CORES)]
    return np.concatenate(shards, axis=0).astype(np.float32)
